# revision 1
# baseline (speedup 1.0000x reference)
"""Trainium2 Bass kernel for the gated-attention nn.Module (v20).

Math (per batch element b):
    deg   = rel_pos.sum(-1)                        # [N]
    gate  = sigmoid(deg * W_d + b_d)               # [N, D]
    xg    = x * gate
    qkv   = xg @ W_qkv.T + b_qkv                   # [N, 3D]
    qk, value, res = split(qkv); qk = sigmoid(qk)
    attn  = (qk @ qk.T) * scale * rel_pos          # [N, N]
    attn  = attn / (attn.sum(-1, keepdims) + 1e-6)
    out   = relu(attn @ value + res)               # [N, D]

Sharding: pure data-parallel over batch, B == 8 == n_cores, one batch
element per NeuronCore, no collectives.

v4 design:
  * rel_pos streams (SWDGE f32->bf16 cast) into 16 RESIDENT tiles that
    are then rewritten IN PLACE, block by block, to attn = scale*S*rel
    as the staircase-scheduled score matmuls complete (block (i,c) runs
    as soon as qk row-tile i and qk key-chunk c exist), so the PE works
    through most of the N^2 scores while rel_pos is still streaming.
  * DMA transposes are mutually exclusive with all other DMA traffic
    (tile serializes xbar-mode transitions), so ALL XBARs are batched
    after the stream, pipelined row-by-row with the attn @ value
    matmuls; output stores are emitted on the same (sync) queue after
    the last XBAR so engine order prevents mode thrash.
  * row-sum normalization is folded into the attn @ value matmul as a
    257th all-ones column of value; z = po[:, 256] is free.
"""

import math
from contextlib import ExitStack

import numpy as np

import concourse.bass as bass
import concourse.tile as tile
from concourse import bacc, mybir
from concourse.bass import ts
from concourse.bass_utils import run_bass_kernel_spmd
from concourse.masks import make_identity

B, N, D = 8, 2048, 256
E = 3 * D  # 768
NT = N // 128  # 16 row tiles
DC = D // 128  # 2 dim chunks
NC = N // 512  # 4 chunks of 512 along the key dim
SCALE = 1.0 / math.sqrt(32.0)
EPS = 1e-6

F32 = mybir.dt.float32
BF16 = mybir.dt.bfloat16

AL = mybir.AluOpType
AF = mybir.ActivationFunctionType


def build_kernel(ctx: ExitStack, tc: tile.TileContext, io: dict):
    nc = tc.nc
    x_d = io["x"]          # [N, D]   f32
    rel_d = io["rel_pos"]  # [N, N]   f32
    wq_d = io["W_qkv"]     # [E, D]   f32
    bq_d = io["b_qkv"]     # [E]      f32
    wd_d = io["W_d"]       # [D, 1]   f32
    bd_d = io["b_d"]       # [D]      f32
    out_d = io["out"]      # [N, D]   f32

    # ---------------- pools ----------------
    consts = ctx.enter_context(tc.tile_pool(name="consts", bufs=1))
    resid = ctx.enter_context(tc.tile_pool(name="resid", bufs=1))
    xpool = ctx.enter_context(tc.tile_pool(name="xpool", bufs=2))
    wk = ctx.enter_context(tc.tile_pool(name="wk", bufs=2))
    small = ctx.enter_context(tc.tile_pool(name="small", bufs=4))
    ptpool = ctx.enter_context(tc.tile_pool(name="ptpool", bufs=6))
    ps = ctx.enter_context(tc.tile_pool(name="ps", bufs=2, space="PSUM"))
    pap = ctx.enter_context(tc.tile_pool(name="pap", bufs=2, space="PSUM"))
    pso = ctx.enter_context(tc.tile_pool(name="pso", bufs=2, space="PSUM"))

    # ---------------- resident tensors ----------------
    # rb2[c][:, h, :] holds rel_pos rows of tile 2c+h (bf16), later
    # rewritten in place to attn = scale * S * rel, block by block.
    rb2 = [
        resid.tile([128, 2, N], BF16, tag=f"rb2_{c}", name=f"rb2_{c}")
        for c in range(NT // 2)
    ]
    rb = [rb2[i // 2][:, i % 2, :] for i in range(NT)]
    qkT = [resid.tile([128, N], BF16, tag=f"qkT{dc}", name=f"qkT{dc}") for dc in range(DC)]
    xgT = [resid.tile([128, N], BF16, tag=f"xgT{dc}", name=f"xgT{dc}") for dc in range(DC)]
    # value rows + a 257th all-ones column (row-sum trick)
    vpx = resid.tile([128, NT, 272], BF16)
    res = [resid.tile([128, D], F32, tag=f"res{j}", name=f"res{j}") for j in range(NT)]
    deg = resid.tile([128, NT], F32)
    dscr = resid.tile([128, N], F32)
    o_all = resid.tile([128, NT, D], F32)

    # ---------------- constants ----------------
    ident = consts.tile([128, 128], BF16)
    make_identity(nc, ident)

    wd_bc = consts.tile([128, D], F32)
    nc.scalar.dma_start(
        out=wd_bc,
        in_=bass.AP(tensor=wd_d.tensor, offset=wd_d.offset, ap=[[0, 128], [1, D]]),
    )
    bd_bc = consts.tile([128, D], F32)
    nc.scalar.dma_start(
        out=bd_bc,
        in_=bass.AP(tensor=bd_d.tensor, offset=bd_d.offset, ap=[[0, 128], [1, D]]),
    )

    ones_row = consts.tile([1, 512], BF16)
    nc.vector.memset(ones_row, 1.0)
    bq_row_f = consts.tile([1, E], F32)
    nc.scalar.dma_start(
        out=bq_row_f,
        in_=bass.AP(tensor=bq_d.tensor, offset=bq_d.offset, ap=[[1, 1], [1, E]]),
    )
    bq_row = consts.tile([1, E], BF16)
    nc.vector.tensor_copy(out=bq_row, in_=bq_row_f)

    # ones column of vpx (row-sum trick), set once
    nc.vector.memset(vpx[:, :, 256:257], 1.0)

    # ---------------- preload big inputs, then the rel_pos stream --------
    # W_qkv and x go through the SWDGE queue FIRST: the in-order queue
    # guarantees they land before the rel stream starts hogging HBM
    # (HWDGE transfers only trickle once the SWDGE stream is running).
    rel_src = rel_d.rearrange("(c h p) k -> c p h k", p=128, h=2)
    nc.gpsimd.dma_start(out=rb2[0], in_=rel_src[0])
    wq_nat, free_wq_nat = tc.tile([128, 6, D], F32, name="wq_nat")
    nc.gpsimd.dma_start(out=wq_nat, in_=wq_d.rearrange("(c p) d -> p c d", p=128))
    xt4 = [xpool.tile([128, 4, D], F32, tag="xt4", name=f"xt4_{g}") for g in range(4)]
    for g in range(4):
        nc.gpsimd.dma_start(
            out=xt4[g], in_=x_d.rearrange("(g q p) d -> g p q d", p=128, q=4)[g]
        )
    for c in range(1, NT // 2):
        nc.gpsimd.dma_start(out=rb2[c], in_=rel_src[c])

    # ---------------- pass A stages ----------------
    def ea_deg(i):
        # split the row-sum between the scalar engine (activation with
        # accumulate) and DVE so neither conveyor saturates
        if i % 8 < 5:
            nc.scalar.activation(
                out=dscr, in_=rb[i], func=AF.Copy, accum_out=deg[:, i : i + 1]
            )
        else:
            nc.vector.tensor_reduce(
                out=deg[:, i : i + 1], in_=rb[i], axis=mybir.AxisListType.X, op=AL.add
            )

    def ea_gate(i):
        gate = wk.tile([128, D], F32, tag="gate", name="gate")
        nc.vector.scalar_tensor_tensor(
            out=gate,
            in0=wd_bc,
            scalar=deg[:, i : i + 1],
            in1=bd_bc,
            op0=AL.mult,
            op1=AL.add,
        )
        nc.scalar.activation(out=gate, in_=gate, func=AF.Sigmoid)
        xg = wk.tile([128, D], BF16, tag="xg", name="xg")
        nc.vector.tensor_tensor(out=xg, in0=xt4[i // 4][:, i % 4, :], in1=gate, op=AL.mult)
        for dc in range(DC):
            pt = ps.tile([128, 128], BF16, tag="ps", name="pt_xg", padded_shape=[128, 1024])
            nc.tensor.transpose(pt, xg[:, ts(dc, 128)], ident)
            nc.scalar.copy(out=xgT[dc][:, ts(i, 128)], in_=pt)

    def ea_vr(i):
        # value / res projection for row-tile i
        pv = ps.tile([128, 512], F32, tag="ps", name="pv")
        for dc in range(DC):
            nc.tensor.matmul(
                pv,
                lhsT=xgT[dc][:, ts(i, 128)],
                rhs=wqT[dc][:, D : 3 * D],
                start=(dc == 0),
                stop=False,
            )
        nc.tensor.matmul(
            pv, lhsT=ones_row[:, 0:128], rhs=bq_row[:, D : 3 * D], start=False, stop=True
        )
        nc.scalar.copy(out=vpx[:, i, 0:256], in_=pv[:, 0:D])
        nc.scalar.copy(out=res[i], in_=pv[:, D : 2 * D])

    def ea_qk(g):
        # qk projection + sigmoid for chunk g (tiles 4g..4g+3)
        for ec in range(DC):
            pq = ps.tile([128, 512], F32, tag="ps", name="pq")
            for dc in range(DC):
                nc.tensor.matmul(
                    pq,
                    lhsT=wqT[dc][:, ts(ec, 128)],
                    rhs=xgT[dc][:, ts(g, 512)],
                    start=(dc == 0),
                    stop=False,
                )
            nc.tensor.matmul(
                pq,
                lhsT=bq_row[:, ts(ec, 128)],
                rhs=ones_row,
                start=False,
                stop=True,
            )
            nc.scalar.activation(
                out=qkT[ec][:, ts(g, 512)],
                in_=pq,
                func=AF.Sigmoid,
            )

    # ---------------- score pairs (1024-wide, natural orientation) -------
    # pair (i, p): attn rows = queries of tile i, keys 1024p..1024p+1023
    npair = 0

    def b1_pair(i, p, allow_act=True):
        nonlocal npair
        pa = pap.tile([128, 1024], F32, tag="pa", name="pa")
        for half in range(2):
            c = 2 * p + half
            for dc in range(DC):
                nc.tensor.matmul(
                    pa[:, ts(half, 512)],
                    lhsT=qkT[dc][:, ts(i, 128)],
                    rhs=qkT[dc][:, ts(c, 512)],
                    start=(dc == 0),
                    stop=(dc == DC - 1),
                )
        blk = rb[i][:, ts(p, 1024)]
        npair += 1
        if not allow_act or i < 8:
            # direct: (pa * SCALE) * rel on DVE (PSUM f32 operand, 1x mode)
            nc.vector.scalar_tensor_tensor(
                out=blk, in0=pa, scalar=SCALE, in1=blk, op0=AL.mult, op1=AL.mult
            )
        else:
            # via ACT: scaled bf16 copy of pa, then a 2x-mode bf16 multiply
            sb = wk.tile([128, 1024], BF16, tag="sb", name="sb")
            nc.scalar.activation(out=sb, in_=pa, func=AF.Copy, scale=SCALE)
            nc.vector.tensor_tensor(out=blk, in0=sb, in1=blk, op=AL.mult)

    # ---------------- post-stream: transpose + attn @ value --------------
    def b2_xbar(i):
        # PT[p, j, q] = attn[128i+q, 128j+p]
        PT = ptpool.tile([128, NT, 128], BF16, tag="PT", name="PT")
        nc.sync.dma_start(out=PT, in_=rb[i], transpose=True)
        return PT

    def b2_tile(i, PT):
        po = pso.tile([128, 257], F32, tag="po", name="po", padded_shape=[128, 512])
        for j in range(NT):
            nc.tensor.matmul(
                po,
                lhsT=PT[:, j, :],
                rhs=vpx[:, j, 0:257],
                start=(j == 0),
                stop=(j == NT - 1),
            )
        z = small.tile([128, 1], F32, tag="z", name="z")
        nc.vector.tensor_scalar_add(out=z, in0=po[:, 256:257], scalar1=EPS)
        zi = small.tile([128, 1], F32, tag="zi", name="zi")
        nc.vector.reciprocal(out=zi, in_=z)
        nc.vector.scalar_tensor_tensor(
            out=o_all[:, i, :], in0=po[:, 0:D], scalar=zi, in1=res[i],
            op0=AL.mult, op1=AL.add,
        )
        nc.scalar.activation(out=o_all[:, i, :], in_=o_all[:, i, :], func=AF.Relu)

    # ---------------- main loop with staircase scores --------------------
    emitted = set()
    pending = []

    def enqueue_ready_pairs(done_tiles):
        dg = done_tiles // 4  # complete qk chunks
        if dg >= 4:
            return  # final batch handled by the tail
        for p in range(dg // 2):
            for i in range(4 * dg):
                if (i, p) not in emitted:
                    emitted.add((i, p))
                    pending.append((i, p))

    def drain_pending(nmax):
        for _ in range(min(nmax, len(pending))):
            i, p = pending.pop(0)
            b1_pair(i, p, allow_act=False)

    # start the deg/gate conveyor for tile 0 before the W_qkv processing so
    # its DVE/ACT ops are not queued behind the weight-transpose chain
    ea_deg(0)
    ea_gate(0)

    # W_qkv -> bf16 -> PE-transposed WqT[dc] = W_qkv.T chunks
    wqT = [consts.tile([128, E], BF16, tag=f"wqT{dc}", name=f"wqT{dc}") for dc in range(DC)]
    wq_nat_bf, free_wq_nat_bf = tc.tile([128, 6, D], BF16, name="wq_nat_bf")
    nc.vector.tensor_copy(out=wq_nat_bf, in_=wq_nat)
    for c in range(6):
        for dc in range(DC):
            pt = ps.tile([128, 128], BF16, tag="ps", name="pt_w", padded_shape=[128, 1024])
            nc.tensor.transpose(pt, wq_nat_bf[:, c, ts(dc, 128)], ident)
            nc.vector.tensor_copy(out=wqT[dc][:, ts(c, 128)], in_=pt)
    free_wq_nat_bf()
    free_wq_nat()

    LAG_G, LAG_V = 1, 2
    for k in range(1, NT + LAG_V + 1):
        if k < NT:
            ea_deg(k)
        if LAG_G <= k < NT + LAG_G and k - LAG_G >= 1:
            ea_gate(k - LAG_G)
        if LAG_V <= k < NT + LAG_V:
            i = k - LAG_V
            ea_vr(i)
            if i % 4 == 3:
                ea_qk(i // 4)
                enqueue_ready_pairs(i + 1)
        drain_pending(1)
    drain_pending(len(pending))

    # ---------------- tail: finish scores + all XBARs, then attn@value ---
    pts = {}
    for k in range(NT):
        for p in range(2):
            if (k, p) not in emitted:
                emitted.add((k, p))
                b1_pair(k, p)
        pts[k] = b2_xbar(k)
    for k in range(NT):
        b2_tile(k, pts.pop(k))

    # output stores: emitted on the sync queue AFTER all XBARs (engine
    # order keeps the xbar-mode transition count at one), in 4 chunks so
    # early chunks overlap the remaining b2 compute
    for g in range(4):
        nc.sync.dma_start(
            out=out_d.rearrange("(g q p) d -> g p q d", p=128, q=4)[g],
            in_=o_all[:, ts(g, 4), :],
        )


_CACHE: dict = {}


def _get_nc():
    if "nc" in _CACHE:
        return _CACHE["nc"], _CACHE["io"]
    nc = bacc.Bacc("TRN2", target_bir_lowering=False, debug=False)
    io = {
        "x": nc.dram_tensor("x", [N, D], F32, kind="ExternalInput").ap(),
        "rel_pos": nc.dram_tensor("rel_pos", [N, N], F32, kind="ExternalInput").ap(),
        "W_qkv": nc.dram_tensor("W_qkv", [E, D], F32, kind="ExternalInput").ap(),
        "b_qkv": nc.dram_tensor("b_qkv", [E], F32, kind="ExternalInput").ap(),
        "W_d": nc.dram_tensor("W_d", [D, 1], F32, kind="ExternalInput").ap(),
        "b_d": nc.dram_tensor("b_d", [D], F32, kind="ExternalInput").ap(),
        "out": nc.dram_tensor("out", [N, D], F32, kind="ExternalOutput").ap(),
    }
    with tile.TileContext(nc) as tc:
        with ExitStack() as ctx:
            build_kernel(ctx, tc, io)
    nc.compile()
    _CACHE["nc"] = nc
    _CACHE["io"] = io
    return nc, io


def kernel(x, rel_pos, W_qkv, b_qkv, W_d, b_d, **run_kwargs):
    nc, _ = _get_nc()
    x = np.ascontiguousarray(np.asarray(x, dtype=np.float32))
    rel_pos = np.ascontiguousarray(np.asarray(rel_pos, dtype=np.float32))
    W_qkv = np.ascontiguousarray(np.asarray(W_qkv, dtype=np.float32))
    b_qkv = np.ascontiguousarray(np.asarray(b_qkv, dtype=np.float32))
    W_d = np.ascontiguousarray(np.asarray(W_d, dtype=np.float32))
    b_d = np.ascontiguousarray(np.asarray(b_d, dtype=np.float32))
    in_maps = [
        {
            "x": x[b],
            "rel_pos": rel_pos[b],
            "W_qkv": W_qkv,
            "b_qkv": b_qkv,
            "W_d": W_d,
            "b_d": b_d,
        }
        for b in range(B)
    ]
    r = run_bass_kernel_spmd(nc, in_maps, core_ids=list(range(B)), **run_kwargs)
    out = np.stack([r.results[b]["out"] for b in range(B)], axis=0)
    if run_kwargs:
        _CACHE["last_result"] = r
    return out

